# revision 7
# baseline (speedup 1.0000x reference)
"""Trainium2 Bass kernel for nn_Attention_FreMLPs (B=16,T=12,N=2048,D=128).

Strategy: data-parallel over batch (2 batches per NeuronCore, no collectives).
Per batch, everything runs in "T-layout" (feature on partitions, tokens on the
free axis) with float32r matmuls (full PE rate at free>=256, ~1.5e-4 rounding).

Algebraic restructuring (validated vs the jax reference in numpy):
  - conv1x1 biases folded: bd2 -> bq/bk/bv, bv -> bu1; gamma -> Wr1/Wi1;
    beta -> f0-bias rows; br2/bi2 -> single final bias via irfft weights.
  - softmax without max-subtraction (max |logit| ~ 0.2), normalization applied
    after the attention matmul via a PE broadcast of 1/rowsum.
  - rfft/irfft over the tiny t-axis (12) are folded into two block-diagonal
    "mix" matmuls operating on d-subgrouped tiles [(t,j),n] built with
    partition-regrouping DMAs through DRAM scratch; MLP layer 1 is applied
    before the rfft and the irfft before layer 2 (both legal by linearity).
"""
import os
import sys

for _p in ("/opt/trn_rl_repo", "/root/.axon_site/_ro/trn_rl_repo"):
    if os.path.isdir(_p) and _p not in sys.path:
        sys.path.insert(0, _p)

import numpy as np

B, T, N, D = 16, 12, 2048, 128
F, FK, G, NG = 7, 14, 8, 16
NCORES = 8
BL = B // NCORES        # batches per core
NQ, Q = 4, 512          # quarters of the token axis
NH, HW_ = 2, 1024       # halves for attention

_PROG_CACHE = {}


# ----------------------------------------------------------------------------
# host-side constant preparation (numpy only)
# ----------------------------------------------------------------------------

def _host_consts(inp):
    f32 = np.float32
    t = np.arange(T)[:, None]
    f = np.arange(F)[None, :]
    ang = 2 * np.pi * t * f / T
    Cre = np.cos(ang).astype(np.float64)
    Cim = -np.sin(ang).astype(np.float64)
    w = np.ones(F); w[1:6] = 2.0; w /= T
    Dre = (w[None, :] * np.cos(ang)).astype(np.float64)
    Dim = -(w[None, :] * np.sin(ang)).astype(np.float64)

    Wd1 = np.asarray(inp["Wd1"], f32); bd1 = np.asarray(inp["bd1"], f32)
    Wd2 = np.asarray(inp["Wd2"], f32); bd2 = np.asarray(inp["bd2"], f32)
    Wq = np.asarray(inp["Wq"], f32); bq = np.asarray(inp["bq"], f32)
    Wk = np.asarray(inp["Wk"], f32); bk = np.asarray(inp["bk"], f32)
    Wv = np.asarray(inp["Wv"], f32); bv = np.asarray(inp["bv"], f32)
    Wu1 = np.asarray(inp["Wu1"], f32); bu1 = np.asarray(inp["bu1"], f32)
    Wu2 = np.asarray(inp["Wu2"], f32); bu2 = np.asarray(inp["bu2"], f32)
    gamma = np.asarray(inp["gamma"], f32); beta = np.asarray(inp["beta"], f32)
    Wr1 = np.asarray(inp["Wr1"], f32); br1 = np.asarray(inp["br1"], f32)
    Wr2 = np.asarray(inp["Wr2"], f32); br2 = np.asarray(inp["br2"], f32)
    Wi1 = np.asarray(inp["Wi1"], f32); bi1 = np.asarray(inp["bi1"], f32)
    Wi2 = np.asarray(inp["Wi2"], f32); bi2 = np.asarray(inp["bi2"], f32)

    c = {}
    c["wd1t"] = np.ascontiguousarray(Wd1.T)                    # [1536,128]
    c["wd2t"] = np.ascontiguousarray(Wd2.T)
    c["wqt"] = np.ascontiguousarray(Wq.T)
    c["wkt"] = np.ascontiguousarray(Wk.T)
    c["wvt"] = np.ascontiguousarray(Wv.T)
    c["wu1t"] = np.ascontiguousarray(Wu1.T)                    # [128,1536]
    Wu2T = Wu2.T                                               # [1536,1536]
    c["wu2t"] = np.ascontiguousarray(
        Wu2T.reshape(T, D, T, D).transpose(0, 2, 1, 3))        # [k,tau,128,128]
    Wr1g = Wr1 * gamma[None, :]
    Wi1g = Wi1 * gamma[None, :]
    c["wr1gt"] = np.ascontiguousarray(Wr1g.T)
    c["wi1gt"] = np.ascontiguousarray(Wi1g.T)
    c["wr2t"] = np.ascontiguousarray(Wr2.T)
    c["wi2t"] = np.ascontiguousarray(Wi2.T)

    c["bd1"] = bd1.reshape(D, 1)
    c["bqf"] = (bq + Wq @ bd2).reshape(D, 1)
    c["bkf"] = (bk + Wk @ bd2).reshape(D, 1)
    bv_f = bv + Wv @ bd2
    c["bu1f"] = np.ascontiguousarray((bu1 + Wu1 @ bv_f).reshape(T, D).T)  # [128,12]
    c["bu2r"] = bu2.reshape(1, T * D)
    # final bias: sum_f Dre[t,f](br2-bi2) + Dim[t,f](br2+bi2)  -> [T,D] -> [128,12]
    bfin = (Dre.sum(axis=1)[:, None] * (br2 - bi2)[None, :]
            + Dim.sum(axis=1)[:, None] * (br2 + bi2)[None, :])
    c["bfin"] = np.ascontiguousarray(bfin.T.astype(f32))       # [128,12]

    # mix blocks
    CR = np.zeros((T * G, FK * G), np.float64)
    DA = np.zeros((FK * G, T * G), np.float64)
    DB = np.zeros((FK * G, T * G), np.float64)
    for tt in range(T):
        for fk in range(FK):
            ff = fk % F
            cre = Cre[tt, ff] if fk < F else Cim[tt, ff]
            dav = Dre[tt, ff] if fk < F else Dim[tt, ff]
            dbv = Dim[tt, ff] if fk < F else -Dre[tt, ff]
            for j in range(G):
                CR[tt * G + j, fk * G + j] = cre
                DA[fk * G + j, tt * G + j] = dav
                DB[fk * G + j, tt * G + j] = dbv
    c["cr"] = CR.astype(f32)
    c["da"] = DA.astype(f32)
    c["db"] = DB.astype(f32)

    corr_r = T * (Wr1 @ beta)
    corr_i = T * (Wi1 @ beta)
    bias_r = np.zeros((FK * G, NG), f32)
    bias_i = np.zeros((FK * G, NG), f32)
    for g in range(NG):
        for fk in range(FK):
            ff = fk % F
            for j in range(G):
                i = g * G + j
                if fk < F:
                    bias_r[fk * G + j, g] = br1[i] + (corr_r[i] if ff == 0 else 0.0)
                    bias_i[fk * G + j, g] = bi1[i] + (corr_i[i] if ff == 0 else 0.0)
                else:
                    bias_r[fk * G + j, g] = br1[i]
                    bias_i[fk * G + j, g] = bi1[i]
    c["biasr"] = bias_r                                        # [112,16]
    c["biasi"] = bias_i

    # stats / broadcast / misc constants
    em = np.zeros((T, D, T), f32)
    bm = np.zeros((T, T, D), f32)
    for tt in range(T):
        em[tt, :, tt] = 1.0
        bm[tt, tt, :] = 1.0
    c["emats"] = em                                            # [12,128,12]
    c["bmats"] = bm                                            # [12,12,128]
    c["ident"] = np.eye(D, dtype=f32)
    c["ones_col"] = np.ones((D, 1), f32)
    c["ones_row"] = np.ones((1, Q), f32)
    c["ones_1x128"] = np.ones((1, D), f32)
    return c


# ----------------------------------------------------------------------------
# device program
# ----------------------------------------------------------------------------

def _build_program():
    import concourse.bacc as bacc
    import concourse.mybir as mybir
    from concourse import tile

    F32 = mybir.dt.float32
    F32R = mybir.dt.float32r
    AF = mybir.ActivationFunctionType
    ALU = mybir.AluOpType

    nc = bacc.Bacc("TRN2", target_bir_lowering=False, debug=False,
                   num_devices=NCORES)

    def din(name, shape, dt=F32R):
        return nc.dram_tensor(name, list(shape), dt, kind="ExternalInput")

    x_d = din("x", [BL, T, N, D])
    xf_d = din("xf", [BL, T, N, D], F32)
    wd1t_d = din("wd1t", [T * D, D])
    wd2t_d = din("wd2t", [D, D])
    wqt_d = din("wqt", [D, D])
    wkt_d = din("wkt", [D, D])
    wvt_d = din("wvt", [D, D])
    wu1t_d = din("wu1t", [D, T * D])
    wu2t_d = din("wu2t", [T, T, D, D])
    wr1gt_d = din("wr1gt", [D, D])
    wi1gt_d = din("wi1gt", [D, D])
    wr2t_d = din("wr2t", [D, D])
    wi2t_d = din("wi2t", [D, D])
    cr_d = din("cr", [T * G, FK * G])
    da_d = din("da", [FK * G, T * G])
    db_d = din("db", [FK * G, T * G])
    bd1_d = din("bd1", [D, 1], F32)
    bqf_d = din("bqf", [D, 1], F32)
    bkf_d = din("bkf", [D, 1], F32)
    bu1f_d = din("bu1f", [D, T], F32)
    bu2r_d = din("bu2r", [1, T * D])
    bfin_d = din("bfin", [D, T], F32)
    biasr_d = din("biasr", [FK * G, NG], F32)
    biasi_d = din("biasi", [FK * G, NG], F32)
    emats_d = din("emats", [T, D, T])
    bmats_d = din("bmats", [T, T, D])
    ident_d = din("ident", [D, D])
    onesc_d = din("ones_col", [D, 1])
    onesr_d = din("ones_row", [1, Q])
    ones1x128_d = din("ones_1x128", [1, D])

    out_d = nc.dram_tensor("out", [BL, T, N, D], F32, kind="ExternalOutput")

    with tile.TileContext(nc) as tc:
        with (
            tc.tile_pool(name="wpool", bufs=1) as wp,
            tc.tile_pool(name="bpool", bufs=1) as bp,          # batch-persistent
            tc.tile_pool(name="dscr", bufs=2, space="DRAM") as dp,
            tc.tile_pool(name="abscr", bufs=4, space="DRAM") as abp_d,
        ):
            # ---- load weights ----
            wd1t = wp.tile([D, T * D], F32R, tag="wd1t")
            nc.sync.dma_start(out=wd1t[:], in_=wd1t_d[:].rearrange("(t p) c -> p t c", p=D))
            wd2t = wp.tile([D, D], F32R, tag="wd2t")
            nc.sync.dma_start(out=wd2t[:], in_=wd2t_d[:])
            wqt = wp.tile([D, D], F32R, tag="wqt")
            nc.sync.dma_start(out=wqt[:], in_=wqt_d[:])
            wkt = wp.tile([D, D], F32R, tag="wkt")
            nc.sync.dma_start(out=wkt[:], in_=wkt_d[:])
            wvt = wp.tile([D, D], F32R, tag="wvt")
            nc.sync.dma_start(out=wvt[:], in_=wvt_d[:])
            wu1t = wp.tile([D, T * D], F32R, tag="wu1t")
            nc.sync.dma_start(out=wu1t[:], in_=wu1t_d[:])
            wu2t = wp.tile([D, T * T * D], F32R, tag="wu2t")
            nc.sync.dma_start(out=wu2t[:], in_=wu2t_d[:].rearrange("k t p c -> p k t c"))
            wr1gt = wp.tile([D, D], F32R, tag="wr1gt")
            nc.sync.dma_start(out=wr1gt[:], in_=wr1gt_d[:])
            wi1gt = wp.tile([D, D], F32R, tag="wi1gt")
            nc.sync.dma_start(out=wi1gt[:], in_=wi1gt_d[:])
            wr2t = wp.tile([D, D], F32R, tag="wr2t")
            nc.sync.dma_start(out=wr2t[:], in_=wr2t_d[:])
            wi2t = wp.tile([D, D], F32R, tag="wi2t")
            nc.sync.dma_start(out=wi2t[:], in_=wi2t_d[:])
            cr_sb = wp.tile([T * G, FK * G], F32R, tag="cr")
            nc.sync.dma_start(out=cr_sb[:], in_=cr_d[:])
            da_sb = wp.tile([FK * G, T * G], F32R, tag="da")
            nc.sync.dma_start(out=da_sb[:], in_=da_d[:])
            db_sb = wp.tile([FK * G, T * G], F32R, tag="db")
            nc.sync.dma_start(out=db_sb[:], in_=db_d[:])
            bd1_sb = wp.tile([D, 1], F32, tag="bd1")
            nc.sync.dma_start(out=bd1_sb[:], in_=bd1_d[:])
            bqf_sb = wp.tile([D, 1], F32, tag="bqf")
            nc.sync.dma_start(out=bqf_sb[:], in_=bqf_d[:])
            bkf_sb = wp.tile([D, 1], F32, tag="bkf")
            nc.sync.dma_start(out=bkf_sb[:], in_=bkf_d[:])
            bu1f_sb = wp.tile([D, T], F32, tag="bu1f")
            nc.sync.dma_start(out=bu1f_sb[:], in_=bu1f_d[:])
            bu2r_sb = wp.tile([1, T * D], F32R, tag="bu2r")
            nc.sync.dma_start(out=bu2r_sb[:], in_=bu2r_d[:])
            bfin_sb = wp.tile([D, T], F32, tag="bfin")
            nc.sync.dma_start(out=bfin_sb[:], in_=bfin_d[:])
            biasr_sb = wp.tile([FK * G, NG], F32, tag="biasr")
            nc.sync.dma_start(out=biasr_sb[:], in_=biasr_d[:])
            biasi_sb = wp.tile([FK * G, NG], F32, tag="biasi")
            nc.sync.dma_start(out=biasi_sb[:], in_=biasi_d[:])
            emats_sb = wp.tile([D, T * T], F32R, tag="emats")
            nc.sync.dma_start(out=emats_sb[:], in_=emats_d[:].rearrange("t p c -> p t c"))
            bmats_sb = wp.tile([T, T * D], F32R, tag="bmats")
            nc.sync.dma_start(out=bmats_sb[:], in_=bmats_d[:].rearrange("t p c -> p t c"))
            ident_sb = wp.tile([D, D], F32R, tag="ident")
            nc.sync.dma_start(out=ident_sb[:], in_=ident_d[:])
            onesc_sb = wp.tile([D, 1], F32R, tag="onesc")
            nc.sync.dma_start(out=onesc_sb[:], in_=onesc_d[:])
            onesr_sb = wp.tile([1, Q], F32R, tag="onesr")
            nc.sync.dma_start(out=onesr_sb[:], in_=onesr_d[:])
            ones1x128_sb = wp.tile([1, D], F32R, tag="ones1x")
            nc.sync.dma_start(out=ones1x128_sb[:], in_=ones1x128_d[:])

            for b in range(BL):
                oT = bp.tile([D, N], F32R, tag="oT")

                # ======== conv_down + q/k/v ========
                with tc.tile_pool(name=f"cd{b}", bufs=1) as cd:
                  with tc.tile_pool(name=f"cdp{b}", bufs=1, space="PSUM") as cdp:
                    y1ps = cdp.tile([D, N], F32, tag="pbig")
                    for t in range(T):
                        xn = cd.tile([D, N], F32R, tag="xn", bufs=2)
                        nc.sync.dma_start(
                            out=xn[:],
                            in_=x_d[b, t].rearrange("(c p) d -> p c d", p=D))
                        xt = cd.tile([D, N], F32R, tag="xt", bufs=2)
                        for c4 in range(4):
                            trp = cdp.tile([D, Q], F32R, tag="trps", bufs=2)
                            for cc in range(4):
                                nc.tensor.transpose(
                                    trp[:, cc * D:(cc + 1) * D],
                                    xn[:, (c4 * 4 + cc) * D:(c4 * 4 + cc + 1) * D],
                                    ident_sb[:])
                            nc.scalar.activation(
                                xt[:, c4 * Q:(c4 + 1) * Q], trp[:], AF.Copy)
                        for c4 in range(4):
                            nc.tensor.matmul(
                                y1ps[:, c4 * Q:(c4 + 1) * Q],
                                wd1t[:, t * D:(t + 1) * D],
                                xt[:, c4 * Q:(c4 + 1) * Q],
                                start=(t == 0), stop=(t == T - 1))
                    y1 = cd.tile([D, N], F32R, tag="y1")
                    nc.scalar.activation(y1[:], y1ps[:], AF.Relu, bias=bd1_sb[:])

                    yps = cdp.tile([D, N], F32, tag="pbig")
                    for c4 in range(4):
                        nc.tensor.matmul(yps[:, c4 * Q:(c4 + 1) * Q], wd2t[:],
                                         y1[:, c4 * Q:(c4 + 1) * Q],
                                         start=True, stop=True)
                    y_sb = cd.tile([D, N], F32R, tag="y1")
                    nc.scalar.activation(y_sb[:], yps[:], AF.Copy)

                    qT = cd.tile([D, N], F32R, tag="qT")
                    kT = cd.tile([D, N], F32R, tag="kT")
                    for dst, wmat, bias in ((qT, wqt, bqf_sb), (kT, wkt, bkf_sb)):
                        pps = cdp.tile([D, N], F32, tag="pbig")
                        for c4 in range(4):
                            nc.tensor.matmul(pps[:, c4 * Q:(c4 + 1) * Q], wmat[:],
                                             y_sb[:, c4 * Q:(c4 + 1) * Q],
                                             start=True, stop=True)
                        nc.scalar.activation(dst[:], pps[:], AF.Identity, bias=bias[:])

                    v_all = cd.tile([D, N], F32R, tag="vall")
                    for m in range(16):
                        vps = cdp.tile([D, D], F32, tag="vps", bufs=2)
                        nc.tensor.matmul(vps[:], y_sb[:, m * D:(m + 1) * D],
                                         wvt[:], start=True, stop=True)
                        nc.scalar.activation(v_all[:, m * D:(m + 1) * D], vps[:], AF.Copy)

                  # ======== attention (n-halves) ========
                  if True:
                    with tc.tile_pool(name=f"at{b}", bufs=1, space="PSUM") as ap:
                        for h in range(NH):
                            ops = ap.tile([D, HW_], F32, tag="ops")
                            sps = ap.tile([1, HW_], F32, tag="sps")
                            for m in range(16):
                                scp = ap.tile([D, HW_], F32, tag="scp", bufs=2)
                                for j in range(2):
                                    nc.tensor.matmul(
                                        scp[:, j * Q:(j + 1) * Q],
                                        kT[:, m * D:(m + 1) * D],
                                        qT[:, h * HW_ + j * Q:h * HW_ + (j + 1) * Q],
                                        start=True, stop=True)
                                ex = cd.tile([D, HW_], F32R, tag="exp", bufs=2)
                                nc.scalar.activation(ex[:], scp[:], AF.Exp, scale=0.125)
                                for j in range(2):
                                    nc.tensor.matmul(
                                        ops[:, j * Q:(j + 1) * Q],
                                        v_all[:, m * D:(m + 1) * D],
                                        ex[:, j * Q:(j + 1) * Q],
                                        start=(m == 0), stop=(m == 15),
                                        skip_group_check=True)
                                    nc.tensor.matmul(
                                        sps[:, j * Q:(j + 1) * Q],
                                        onesc_sb[:],
                                        ex[:, j * Q:(j + 1) * Q],
                                        start=(m == 0), stop=(m == 15),
                                        skip_group_check=True)
                            ssb = cd.tile([1, HW_], F32, tag="ssb", bufs=2)
                            nc.vector.tensor_copy(ssb[:], sps[:])
                            rec = cd.tile([1, HW_], F32R, tag="rec", bufs=2)
                            with nc.allow_low_precision(reason="f32r rounding intended"):
                                nc.vector.reciprocal(rec[:], ssb[:])
                            rbp = ap.tile([D, HW_], F32, tag="scp", bufs=2)
                            for j in range(2):
                                nc.tensor.matmul(rbp[:, j * Q:(j + 1) * Q],
                                                 ones1x128_sb[:],
                                                 rec[:, j * Q:(j + 1) * Q],
                                                 start=True, stop=True)
                            rbs = cd.tile([D, HW_], F32, tag="rbs", bufs=1)
                            nc.scalar.activation(rbs[:], rbp[:], AF.Copy)
                            nc.vector.tensor_tensor(oT[:, h * HW_:(h + 1) * HW_],
                                                    ops[:], rbs[:], ALU.mult)

                # ======== back half, per quarter ========
                for q in range(NQ):
                    with (
                        tc.tile_pool(name=f"qq{b}_{q}", bufs=1) as qq,
                    ):
                        up1 = qq.tile([D, T * Q], F32R, tag="up1")
                        hn = qq.tile([D, T * Q], F32R, tag="hn")

                        # ---- conv_up ----
                        with tc.tile_pool(name=f"cu{b}{q}", bufs=1, space="PSUM") as cup:
                            for tau in range(T):
                                u1p = cup.tile([D, Q], F32, tag="u1", bufs=2)
                                nc.tensor.matmul(u1p[:],
                                                 wu1t[:, tau * D:(tau + 1) * D],
                                                 oT[:, q * Q:(q + 1) * Q],
                                                 start=True, stop=True)
                                nc.scalar.activation(
                                    up1[:, tau * Q:(tau + 1) * Q], u1p[:], AF.Relu,
                                    bias=bu1f_sb[:, tau:tau + 1])
                            for tau in range(T):
                                u2p = cup.tile([D, Q], F32, tag="u2", bufs=2)
                                for k in range(T):
                                    nc.tensor.matmul(
                                        u2p[:],
                                        wu2t[:, (k * T + tau) * D:(k * T + tau + 1) * D],
                                        up1[:, k * Q:(k + 1) * Q],
                                        start=(k == 0), stop=False)
                                nc.tensor.matmul(
                                    u2p[:], bu2r_sb[:, tau * D:(tau + 1) * D],
                                    onesr_sb[:], start=False, stop=True)
                                # x^T for this (tau, q) just-in-time
                                xqn = qq.tile([D, Q], F32R, tag="b512r", bufs=6)
                                nc.sync.dma_start(
                                    out=xqn[:],
                                    in_=x_d[b, tau, q * Q:(q + 1) * Q, :]
                                        .rearrange("(c p) d -> p c d", p=D))
                                xqt = qq.tile([D, Q], F32R, tag="b512r", bufs=6)
                                trq = cup.tile([D, Q], F32R, tag="trq", bufs=2)
                                for cc in range(4):
                                    nc.tensor.transpose(
                                        trq[:, cc * D:(cc + 1) * D],
                                        xqn[:, cc * D:(cc + 1) * D], ident_sb[:])
                                nc.scalar.activation(xqt[:], trq[:], AF.Copy)
                                nc.vector.tensor_tensor(
                                    hn[:, tau * Q:(tau + 1) * Q], u2p[:], xqt[:],
                                    ALU.add)

                        # ---- LayerNorm over d (partitions) ----
                        with tc.tile_pool(name=f"ln{b}{q}", bufs=1, space="PSUM") as lnp:
                            stp = lnp.tile([T, Q], F32, tag="st")
                            sqp = lnp.tile([T, Q], F32, tag="sq")
                            for tau in range(T):
                                sqt = qq.tile([D, Q], F32R, tag="b512r", bufs=6)
                                nc.scalar.activation(sqt[:], hn[:, tau * Q:(tau + 1) * Q],
                                                     AF.Square)
                                nc.tensor.matmul(stp[:], emats_sb[:, tau * T:(tau + 1) * T],
                                                 hn[:, tau * Q:(tau + 1) * Q],
                                                 start=(tau == 0), stop=(tau == T - 1),
                                                 skip_group_check=True)
                                nc.tensor.matmul(sqp[:], emats_sb[:, tau * T:(tau + 1) * T],
                                                 sqt[:],
                                                 start=(tau == 0), stop=(tau == T - 1),
                                                 skip_group_check=True)
                            mu = qq.tile([T, Q], F32R, tag="mu")
                            nc.vector.tensor_scalar_mul(mu[:], stp[:], 1.0 / D)
                            msq = qq.tile([T, Q], F32, tag="stmp", bufs=3)
                            nc.vector.tensor_scalar_mul(msq[:], sqp[:], 1.0 / D)
                            mu2 = qq.tile([T, Q], F32, tag="stmp", bufs=3)
                            nc.vector.tensor_tensor(mu2[:], mu[:].bitcast(F32), mu[:].bitcast(F32), ALU.mult)
                            var = qq.tile([T, Q], F32, tag="stmp", bufs=3)
                            nc.vector.tensor_tensor(var[:], msq[:], mu2[:], ALU.subtract)
                            vare = qq.tile([T, Q], F32, tag="stmp", bufs=3)
                            nc.vector.tensor_scalar_add(vare[:], var[:], 1e-5)
                            sd = qq.tile([T, Q], F32, tag="stmp", bufs=3)
                            nc.scalar.activation(sd[:], vare[:], AF.Sqrt)
                            rstd = qq.tile([T, Q], F32R, tag="rstd")
                            with nc.allow_low_precision(reason="f32r rounding intended"):
                                nc.vector.reciprocal(rstd[:], sd[:])
                            mur = qq.tile([T, Q], F32R, tag="mur")
                            nc.vector.tensor_tensor(mur[:], mu[:].bitcast(F32), rstd[:].bitcast(F32), ALU.mult)
                            for tau in range(T):
                                abc = lnp.tile([D, Q], F32, tag="abc", bufs=2)
                                nc.tensor.matmul(abc[:], bmats_sb[:, tau * D:(tau + 1) * D],
                                                 rstd[:], start=True, stop=True)
                                ccp = lnp.tile([D, Q], F32, tag="ccp", bufs=2)
                                nc.tensor.matmul(ccp[:], bmats_sb[:, tau * D:(tau + 1) * D],
                                                 mur[:], start=True, stop=True)
                                sl = hn[:, tau * Q:(tau + 1) * Q]
                                nc.vector.tensor_tensor(sl, sl.bitcast(F32), abc[:], ALU.mult)
                                nc.vector.tensor_tensor(sl, sl.bitcast(F32), ccp[:], ALU.subtract)

                        # ---- FFT-MLP section: two streams ----
                        ab_scrs = []
                        for sname, wmat, bias_sb, dmat in (
                            ("r", wr1gt, biasr_sb, da_sb),
                            ("i", wi1gt, biasi_sb, db_sb),
                        ):
                            w_scr = dp.tile([T, D, Q], F32R, tag="wscr")
                            with tc.tile_pool(name=f"ff{b}{q}{sname}", bufs=1,
                                              space="PSUM") as ffp:
                                for tau in range(T):
                                    wps = ffp.tile([D, Q], F32, tag="wp", bufs=2)
                                    nc.tensor.matmul(wps[:], wmat[:],
                                                     hn[:, tau * Q:(tau + 1) * Q],
                                                     start=True, stop=True)
                                    wsb = qq.tile([D, Q], F32R, tag="b512r", bufs=6)
                                    nc.vector.tensor_copy(wsb[:], wps[:])
                                    nc.sync.dma_start(out=w_scr[tau], in_=wsb[:])
                                ab_scr = abp_d.tile([T, D, Q], F32R, tag="ab")
                                ab_scrs.append(ab_scr)
                                for g in range(NG):
                                    grp = qq.tile([T * G, Q], F32R, tag="b512g", bufs=5)
                                    nc.sync.dma_start(
                                        out=grp[:],
                                        in_=w_scr[:, g * G:(g + 1) * G, :])
                                    zps = ffp.tile([FK * G, Q], F32, tag="zp", bufs=2)
                                    nc.tensor.matmul(zps[:], cr_sb[:], grp[:],
                                                     start=True, stop=True)
                                    gel = qq.tile([FK * G, Q], F32R, tag="b512g", bufs=5)
                                    nc.scalar.activation(gel[:], zps[:], AF.Gelu,
                                                         bias=bias_sb[:, g:g + 1])
                                    abps = ffp.tile([T * G, Q], F32, tag="abp", bufs=2)
                                    nc.tensor.matmul(abps[:], dmat[:], gel[:],
                                                     start=True, stop=True)
                                    absb = qq.tile([T * G, Q], F32R, tag="b512g", bufs=5)
                                    nc.vector.tensor_copy(absb[:], abps[:])
                                    nc.sync.dma_start(
                                        out=ab_scr[:, g * G:(g + 1) * G, :],
                                        in_=absb[:])

                        # ---- layer 2 + irfft-bias + transpose + residual + out ----
                        a_scr, b_scr = ab_scrs
                        with tc.tile_pool(name=f"l2{b}{q}", bufs=1, space="PSUM") as l2p:
                            for tau in range(T):
                                a_t = qq.tile([D, Q], F32R, tag="b512r", bufs=6)
                                nc.sync.dma_start(out=a_t[:], in_=a_scr[tau])
                                b_t = qq.tile([D, Q], F32R, tag="b512r", bufs=6)
                                nc.sync.dma_start(out=b_t[:], in_=b_scr[tau])
                                o2p = l2p.tile([D, Q], F32, tag="o2", bufs=2)
                                nc.tensor.matmul(o2p[:], wr2t[:], a_t[:],
                                                 start=True, stop=False)
                                nc.tensor.matmul(o2p[:], wi2t[:], b_t[:],
                                                 start=False, stop=True)
                                o2s = qq.tile([D, Q], F32R, tag="b512r", bufs=6)
                                nc.scalar.activation(o2s[:], o2p[:], AF.Identity,
                                                     bias=bfin_sb[:, tau:tau + 1])
                                tr2 = l2p.tile([D, Q], F32R, tag="tr2", bufs=2)
                                for cc in range(4):
                                    nc.tensor.transpose(tr2[:, cc * D:(cc + 1) * D],
                                                        o2s[:, cc * D:(cc + 1) * D],
                                                        ident_sb[:])
                                xres = qq.tile([D, Q], F32, tag="b512f", bufs=4)
                                nc.sync.dma_start(
                                    out=xres[:],
                                    in_=xf_d[b, tau, q * Q:(q + 1) * Q, :]
                                        .rearrange("(c p) d -> p c d", p=D))
                                ob = qq.tile([D, Q], F32, tag="b512f", bufs=4)
                                nc.vector.tensor_tensor(ob[:], tr2[:].bitcast(F32),
                                                        xres[:], ALU.add)
                                nc.sync.dma_start(
                                    out=out_d[b, tau, q * Q:(q + 1) * Q, :]
                                        .rearrange("(c p) d -> p c d", p=D),
                                    in_=ob[:])
    nc.compile()
    return nc


def _get_program():
    if "nc" not in _PROG_CACHE:
        _PROG_CACHE["nc"] = _build_program()
    return _PROG_CACHE["nc"]


# ----------------------------------------------------------------------------
# runner
# ----------------------------------------------------------------------------

def run_sharded(inputs, trace=False, tmpdir=None):
    from concourse.bass_utils import run_bass_kernel_spmd

    nc = _get_program()
    c = _host_consts(inputs)
    x = np.ascontiguousarray(np.asarray(inputs["x"], np.float32))
    base = {k: np.ascontiguousarray(v.astype(np.float32)) for k, v in c.items()}
    in_maps = []
    for i in range(NCORES):
        m = dict(base)
        xs = np.ascontiguousarray(x[i * BL:(i + 1) * BL])
        m["x"] = xs
        m["xf"] = xs
        in_maps.append(m)
    res = run_bass_kernel_spmd(nc, in_maps, list(range(NCORES)), trace=trace,
                               tmpdir=tmpdir)
    out = np.concatenate([res.results[i]["out"] for i in range(NCORES)], axis=0)
    return out, res.exec_time_ns


def kernel(**inputs):
    return run_sharded(inputs)[0]
